# revision 48
# baseline (speedup 1.0000x reference)
"""Trainium2 Bass kernel for DynamicViewSampler.

Per sample b (of B=16): spotlight weights m[v,l] = exp(-20*dist2(center_v,
coord_l)) * (l < v_len[b]); out[b,v,:] = (m @ v_pad[b]) / (sum_l m + 1e-6).

Strategy (ragged_sequence): m is exactly 0 for l >= v_len[b], so only
ceil(v_len[b]/128) l-tiles of work exist per sample.  The host packs the
valid 128-row l-tiles into per-core groups (the single SPMD program is
identical across the 8 cores; all per-core variation is carried by the
packed input data).

Three dtype tiers, chosen per sample by v_len (error analysis: the
output is a weighted average over ~0.08*v_len tokens, so quantization
noise in v and m averages out for long samples but shows through for
short ones; measured worst-sample rel err 1.15e-2 vs the 2e-2 gate):
  - LONG (v_len >= 1024): v cast to fp8e4m3, m produced in fp8 by the
    ACT exp, and the numerator matmuls run in DoubleRow perf mode
    (2 fp8 weights per PE cell -> one N=512 matmul contracts TWO 128-row
    l-tiles).  Halves both the HBM traffic and the PE time vs bf16.
  - MID (512 <= v_len < 1024): fp8 m is still fine but fp8 v is not, so
    each l-tile becomes a (hi, lo) fp8 pair (lo = fp8(v - hi), together
    bf16-quality) contracted in one DoubleRow matmul with IDENTICAL m
    columns.  A packed den-weight input (1 for hi, 0 for lo k-tiles)
    replaces the all-ones den rhs so den counts m once -- data-driven,
    so the SPMD program stays identical across cores.
  - SHORT (v_len < 512): bf16 v and m, plain matmuls, packed into
    single-tile groups (so the padding DMA that SPMD forces on the
    other cores stays tiny).

On device, per group (layout: l on partitions):
  - tiny K=4 fp32 matmul per k-tile: psc[l,v] = x_l*cx_v + y_l*cy_v
    - (cx_v^2+cy_v^2)/2 - (x_l^2+y_l^2)/2  (rows: x, y, 1, bias; the
    bias row is -1e5/40 for invalid/padding rows -> m = exp(-1e5) = 0,
    which realizes the ragged mask and all padding).  Kept fp32: short-
    sample softmax gaps are sensitive to coordinate quantization.
  - one batched ACT: m[l,t,v] = Exp(40*psc) -> fp8 (long/mid) / bf16
  - numerator: psum[v,d] += sum_i m[l,2j+i,v].T @ v[l,2j+i,d] via
    DoubleRow pairs; den rides as psum column D (rhs = den weights).
    An odd-sized slot finishes with one plain fp8 matmul.
accumulated in PSUM over the k-tiles of a group (one group = one
contiguous chunk of one sample).  Scheduling: group 0's v-load is issued
first so the HBM stream (the bottleneck: ~13.4us/core of fp8 v data)
starts immediately; cw/dw constants ride the scalar HWDGE ring; psum
drains to a bf16 SBUF stage (DVE) and stores ride the SWDGE queue in
2-group batches, except the last groups which use the by-then-idle HWDGE
rings (the drain tail gates the end-to-end time).  The last group runs
its matmuls h-outer so each psum half can drain while the other half is
still accumulating.  Host sums the per-group partials and divides.
"""

import math

import numpy as np
import ml_dtypes

GAMMA = 20.0
P = 128
NCORES = 8
NEG_BIG = -1e5  # exp(40*psum + NEG_BIG) == 0.0 in fp32 for any |psum| ~ O(1)
FP8_MIN_LEN = 1024   # samples at least this long ride the plain fp8 tier
HILO_MIN_LEN = 512   # [HILO_MIN, FP8_MIN): fp8 hi+lo split (bf16-quality v,
                     # fp8 m); each l-tile becomes a (hi, lo) DoubleRow pair
                     # with den-weights (1, 0) so den counts m only once

# knobs (test.py may override)
REPLICAS = 1          # >1: repeat the whole compute for differential timing
LOOP_N = 1            # >1: wrap the body in a hardware For_i loop (timing)
S8 = 8                # fp8 slot size (even; 8*64*4B = exactly one PSUM bank)
OUT_F32 = False       # numerator partials dtype (bf16 halves out-DMA)
VBUFS = 4             # v-data prefetch depth
COPY_SPLIT = False    # psum->stage copies split DVE+Pool (else DVE only)
STORE_MODE = "swdge2"  # "swdge" | "sync" | "swdge2" (2-group batched)
LAST_SPLIT = True     # last group: two half-stores on the idle HWDGE rings
BF_FIRST = True       # bf16 singleton group first (else last)

LAST = {}             # debug/timing info from the most recent kernel() call

_BUILD_CACHE = {}


# ----------------------------------------------------------------- planning

def _eff_grid(v_len, grid_thws):
    """Replicate reference W_eff/H_eff in float32-exact numpy."""
    Lv = v_len.astype(np.float32)
    H = grid_thws[:, 1].astype(np.float32)
    W = grid_thws[:, 2].astype(np.float32)
    W_eff = np.maximum(1, np.round(np.sqrt(Lv * (W / H))).astype(np.int32))
    H_eff = np.maximum(
        1, np.ceil(Lv / W_eff.astype(np.float32)).astype(np.int32)
    )
    return W_eff, H_eff


def _assign(nt, samples, szs, even_only=None):
    """Best-fit chunks of `samples` (8 core-positions per slot), or None.

    Samples in `even_only` (hilo: k-tiles must pair as hi/lo) may only
    use even-sized slots and always take an even count.
    """
    even_only = even_only or set()
    free = {g: NCORES for g in range(len(szs))}
    placed = {g: [] for g in range(len(szs))}
    order = sorted(samples, key=lambda b: -nt[b])
    for b in order:
        n = int(nt[b])
        ev = b in even_only
        k0 = 0
        while k0 < n:
            rem = n - k0
            ok = [g for g in free if free[g] > 0
                  and not (ev and szs[g] % 2)]
            fits = [g for g in ok if szs[g] >= rem]
            if fits:
                g = min(fits, key=lambda g: (szs[g], g))  # tightest fit
            else:
                if not ok:
                    return None
                g = max(ok, key=lambda g: szs[g])  # biggest, partial
            take = min(szs[g], rem)
            if ev:
                take -= take % 2
            if take == 0:
                return None
            placed[g].append((int(b), k0, take))
            free[g] -= 1
            k0 += take
    out = [[None] * len(szs) for _ in range(NCORES)]
    for g, chunks in placed.items():
        for c, grp in enumerate(chunks):
            out[c][g] = grp
    return out


def _plan(v_len):
    """Choose static per-slot sizes/dtypes and assign sample tile-chunks.

    fp8-tier chunk units are k-tiles: 1 per l-tile for plain-fp8 samples,
    2 per l-tile (hi, lo) for hilo samples.  Both are even-aligned inside
    even-sized slots, so every DoubleRow pair is (2j, 2j+1).
    """
    B = len(v_len)
    nt = np.maximum(1, (np.asarray(v_len).astype(np.int64) + P - 1) // P)
    longs = [b for b in range(B) if v_len[b] >= FP8_MIN_LEN]
    hilos = [b for b in range(B)
             if HILO_MIN_LEN <= v_len[b] < FP8_MIN_LEN]
    shorts = [b for b in range(B) if v_len[b] < HILO_MIN_LEN]
    nt8 = nt.copy()
    nt8[hilos] *= 2  # k-tiles
    f8_samples = longs + hilos
    total8 = int(nt8[f8_samples].sum()) if f8_samples else 0
    total16 = int(nt[shorts].sum()) if shorts else 0

    # fp8 tier: even slot sizes ([S8]*k + an even tail), cheapest feasible
    sizes8, slots8 = [], [[] for _ in range(NCORES)]
    if f8_samples:
        capmin = (total8 + NCORES - 1) // NCORES
        # size multisets from {2..8} with at most one odd slot (an odd
        # slot's last k-tile runs as a single non-DoubleRow matmul) and
        # capacity in [capmin, capmin+8]; cheapest first
        cands = []

        def _gen(prefix, rem, maxsz):
            cap = sum(prefix)
            nodd = sum(s % 2 for s in prefix)
            if nodd > 1:
                return
            if cap >= capmin:
                cands.append((cap * 364 + len(prefix) * 550, list(prefix)))
            if rem == 0 or cap > capmin + S8:
                return
            for s in range(min(S8, maxsz), 1, -1):
                _gen(prefix + [s], rem - 1, s)

        _gen([], 10, S8)
        cands.sort(key=lambda c: c[0])
        for _cost, cand in cands:
            sl = _assign(nt8, f8_samples, cand, even_only=set(hilos))
            if sl is not None:
                sizes8, slots8 = cand, sl
                break
        assert sizes8, "fp8 slot assignment failed"

    # bf16 tier: singleton slots
    n16 = (total16 + NCORES - 1) // NCORES if shorts else 0
    sizes16 = [1] * n16
    slots16 = (_assign(nt, shorts, sizes16) if n16 else
               [[] for _ in range(NCORES)])
    assert slots16 is not None, "bf16 slot assignment failed"

    # program order: one bf16 group first (small first DMA -> PE starts
    # early), then the fp8 groups biggest-first, then remaining bf16.
    groups = []   # (tier, sz, tier_slot_idx)
    nlead = min(1, n16) if BF_FIRST else 0
    if nlead:
        groups.append(("bf", 1, 0))
    order8 = sorted(range(len(sizes8)), key=lambda g: -sizes8[g])
    for g in order8:
        groups.append(("f8", sizes8[g], g))
    for g in range(nlead, n16):
        groups.append(("bf", 1, g))

    G = len(groups)
    slots = [[None] * G for _ in range(NCORES)]
    for c in range(NCORES):
        for gi, (tier, _sz, tg) in enumerate(groups):
            src = slots8 if tier == "f8" else slots16
            if tg < len(src[c]):
                slots[c][gi] = src[c][tg]

    # tile offsets: within each tier's v buffer, and global (for cw cols)
    toff8, toff16, gtoff = [], [], []
    a8 = a16 = at = 0
    for tier, sz, _tg in groups:
        gtoff.append(at)
        at += sz
        if tier == "f8":
            toff8.append(a8)
            a8 += sz
        else:
            toff16.append(a16)
            a16 += sz
    ttoff = []
    i8 = i16 = 0
    for tier, sz, _tg in groups:
        if tier == "f8":
            ttoff.append(toff8[i8]); i8 += 1
        else:
            ttoff.append(toff16[i16]); i16 += 1

    plan = {
        "groups": groups, "slots": slots, "G": G,
        "TT8": a8, "TT16": a16, "TTall": at,
        "ttoff": ttoff, "gtoff": gtoff,
        "total": int(nt.sum()), "total8": total8, "total16": total16,
        "sizes": [sz for _t, sz, _g in groups],
        "hilo": set(hilos),
    }
    return plan


# ------------------------------------------------------------- host packing

def _pack(v_pad, v_len, grid_thws, centers, plan):
    B, L, D = v_pad.shape
    V = centers.shape[1]
    groups, slots, G = plan["groups"], plan["slots"], plan["G"]
    TT8, TT16, TTall = plan["TT8"], plan["TT16"], plan["TTall"]
    ttoff, gtoff = plan["ttoff"], plan["gtoff"]
    W_eff, H_eff = _eff_grid(v_len, grid_thws)

    f8 = ml_dtypes.float8_e4m3
    bf = ml_dtypes.bfloat16
    hilo = plan["hilo"]
    vq = {}  # per-sample quantized v (cast once, shared across cores)

    def getv(b, tier):
        key = (b, tier)
        if key not in vq:
            hi = min(L, int(math.ceil(v_len[b] / P)) * P)
            if tier == "bf":
                vq[key] = v_pad[b, :hi].astype(bf)
            elif b in hilo:
                vhi = v_pad[b, :hi].astype(f8)
                vlo = (v_pad[b, :hi] - vhi.astype(np.float32)).astype(f8)
                vq[key] = (vhi, vlo)
            else:
                vq[key] = v_pad[b, :hi].astype(f8)
        return vq[key]

    in_maps = []
    for c in range(NCORES):
        vp8 = np.zeros(P * TT8 * D, dtype=f8)
        vp16 = np.zeros(P * TT16 * D, dtype=bf)
        # den weights, one column per fp8-tier k-tile: 1 everywhere except
        # the lo halves of hilo pairs (den must count m once per l-tile)
        dw = np.ones((P, max(1, TT8)), dtype=f8)
        cw = np.zeros((4, TTall * P + G * V), dtype=np.float32)
        cw[3, :TTall * P] = np.float32(NEG_BIG / (2 * GAMMA))  # mask default
        cr = cw[:, TTall * P:]
        cr[3, :] = 1.0  # bias row coefficient (also masks dummy groups)
        for g, (tier, sz, _tg) in enumerate(groups):
            slot = slots[c][g]
            if slot is None:
                continue
            b, k0, n_real = slot
            is_hilo = tier == "f8" and b in hilo
            cx = centers[b, :, 0].astype(np.float32)
            cy = centers[b, :, 1].astype(np.float32)
            cr[0, g * V:(g + 1) * V] = cx
            cr[1, g * V:(g + 1) * V] = cy
            cr[2, g * V:(g + 1) * V] = -(cx * cx + cy * cy) / np.float32(2.0)
            We = np.int32(W_eff[b])
            He_f = np.float32(H_eff[b])
            We_f = np.float32(We)
            to = ttoff[g]
            vbuf = vp8 if tier == "f8" else vp16
            block = vbuf[P * to * D:P * (to + sz) * D].reshape(P, sz * D)
            vs = getv(b, tier)
            for j in range(n_real):
                t = gtoff[g] + j
                if is_hilo:
                    k = (k0 + j) // 2       # original l-tile index
                    block[:, j * D:(j + 1) * D] = (
                        vs[(k0 + j) % 2][k * P:(k + 1) * P, :])
                    if (k0 + j) % 2:
                        dw[:, to + j] = 0.0
                else:
                    k = k0 + j
                    block[:, j * D:(j + 1) * D] = vs[k * P:(k + 1) * P, :]
                l = np.arange(k * P, (k + 1) * P, dtype=np.int32)
                x = (l % We).astype(np.float32) / We_f
                y = (l // We).astype(np.float32) / He_f
                cw[0, t * P:(t + 1) * P] = x
                cw[1, t * P:(t + 1) * P] = y
                cw[2, t * P:(t + 1) * P] = 1.0
                valid = l < v_len[b]
                bias = -(x * x + y * y) / np.float32(2.0)
                cw[3, t * P:(t + 1) * P] = np.where(
                    valid, bias.astype(np.float32),
                    np.float32(NEG_BIG / (2 * GAMMA)))
        in_maps.append({"vp8": vp8, "vp16": vp16, "cw": cw, "dw": dw})
    return in_maps


# ------------------------------------------------------------ device kernel

def _build(plan, D, V, replicas):
    groups, G = plan["groups"], plan["G"]
    TT8, TT16, TTall = plan["TT8"], plan["TT16"], plan["TTall"]
    ttoff, gtoff = plan["ttoff"], plan["gtoff"]
    key = (tuple((t, s) for t, s, _g in groups), D, V, replicas, OUT_F32,
           LOOP_N, VBUFS, COPY_SPLIT, STORE_MODE, LAST_SPLIT)
    if key in _BUILD_CACHE:
        return _BUILD_CACHE[key]

    import concourse.bass as bass  # noqa: F401
    import concourse.tile as tile
    from concourse import bacc, mybir

    f32 = mybir.dt.float32
    bf16 = mybir.dt.bfloat16
    fp8 = mybir.dt.float8e4
    out_dt = f32 if OUT_F32 else bf16
    DoubleRow = mybir.MatmulPerfMode.DoubleRow

    nc = bacc.Bacc("TRN2", target_bir_lowering=False, debug=False,
                   num_devices=NCORES)
    vp8 = nc.dram_tensor("vp8", [max(1, P * TT8 * D)], fp8,
                         kind="ExternalInput")
    vp16 = nc.dram_tensor("vp16", [max(1, P * TT16 * D)], bf16,
                          kind="ExternalInput")
    cw = nc.dram_tensor("cw", [4, TTall * P + G * V], f32,
                        kind="ExternalInput")
    dw = nc.dram_tensor("dw", [P, max(1, TT8)], fp8, kind="ExternalInput")
    # den rides as column D of each group's block: one copy, one store
    on = nc.dram_tensor("on", [V, G * (D + 1)], out_dt, kind="ExternalOutput")

    Exp = mybir.ActivationFunctionType.Exp

    with tile.TileContext(nc) as tc:
        with (
            tc.tile_pool(name="singles", bufs=1) as singles,
            tc.tile_pool(name="vpool", bufs=VBUFS) as vpool,
            tc.tile_pool(name="mpool", bufs=3) as mpool,
            tc.tile_pool(name="stage", bufs=3) as stpool,
            tc.tile_pool(name="psc", bufs=2, space="PSUM") as psc,
            tc.tile_pool(name="psm", bufs=2, space="PSUM") as psm,
        ):
            # group 0's v load goes FIRST so the HBM stream (the kernel's
            # critical resource) starts as early as possible; cw/dw slot
            # into the descriptor-generation shadow of the next loads
            tier0, sz0, _tg0 = groups[0]
            vg0 = vpool.tile([P, sz0, D], fp8 if tier0 == "f8" else bf16)
            src0 = (vp8 if tier0 == "f8" else vp16)[
                0:P * sz0 * D].rearrange("(p t d) -> p t d", p=P, t=sz0)
            nc.sync.dma_start(vg0, src0)
            # cw/dw gens ride the scalar ring so the sync ring's gen
            # pipeline stays dedicated to the v-load stream
            cw_sb = singles.tile([4, TTall * P + G * V], f32)
            nc.scalar.dma_start(cw_sb, cw[:, :])
            dw_sb = singles.tile([P, max(1, TT8), 1], fp8)
            nc.scalar.dma_start(dw_sb, dw[:, :].rearrange(
                "p (t one) -> p t one", one=1))
            cr_sb = cw_sb[:, TTall * P:]
            ones16 = singles.tile([P, 1], bf16)
            nc.any.memset(ones16, 1.0)
            # warm the ACT exp table (1.3us load) off the critical path
            warm = singles.tile([1, 1], f32)
            nc.any.memset(warm, 0.0)
            nc.scalar.activation(warm, warm, Exp)

            import contextlib
            loop_ctx = (
                tc.For_i(0, LOOP_N, 1,
                         hint_engines=(mybir.EngineType.PE,
                                       mybir.EngineType.SP,
                                       mybir.EngineType.Activation,
                                       mybir.EngineType.DVE))
                if LOOP_N > 1 else contextlib.nullcontext()
            )
            with loop_ctx:
              for _r in range(replicas):
                for g, (tier, sz, _tg) in enumerate(groups):
                    is8 = tier == "f8"
                    vdt = fp8 if is8 else bf16
                    to = ttoff[g]
                    if g == 0 and _r == 0:
                        vg = vg0
                    else:
                        vg = vpool.tile([P, sz, D], vdt)
                        vsrc = vp8 if is8 else vp16
                        src = vsrc[P * to * D:
                                   P * (to + sz) * D].rearrange(
                            "(p t d) -> p t d", p=P, t=sz)
                        nc.sync.dma_start(vg, src)
                    # one psum tile holds the whole group's coords dots
                    # (bias folded in as the 4th contraction row), exp'd in
                    # a single batched ACT instruction
                    ps_c = psc.tile([P, sz, V], f32)
                    for j in range(sz):
                        t = gtoff[g] + j
                        nc.tensor.matmul(
                            ps_c[:, j, :],
                            lhsT=cw_sb[:, t * P:(t + 1) * P],
                            rhs=cr_sb[:, g * V:(g + 1) * V],
                            start=True, stop=True,
                        )
                    m_all = mpool.tile([P, sz, V], vdt)
                    nc.scalar.activation(m_all, ps_c, Exp, scale=2.0 * GAMMA)
                    # m is the stationary operand: one LDW per (pair of)
                    # tile(s), v streams through as two N=512 matmuls.
                    # num in banks 0-1, den column in bank 2 — three
                    # concurrently-pending accumulation groups in distinct
                    # zero-regions.
                    ps_main = psm.tile([V, D + 1], f32)
                    if is8:
                        np_ = sz // 2
                        if g == G - 1 and sz % 2 == 0:
                            # h-outer for the drain group: the [0:512] psum
                            # bank finishes all its accumulation early, so
                            # its copy+store overlap the second half's
                            # matmuls (costs one extra LDW per pair)
                            for h in range(D // 512):
                                for jp in range(np_):
                                    nc.tensor.matmul(
                                        ps_main[:, h * 512:(h + 1) * 512],
                                        lhsT=m_all[:, 2 * jp:2 * jp + 2, :],
                                        rhs=vg[:, 2 * jp:2 * jp + 2,
                                               h * 512:(h + 1) * 512],
                                        start=jp == 0, stop=jp == np_ - 1,
                                        perf_mode=DoubleRow,
                                    )
                            for jp in range(np_):
                                nc.tensor.matmul(
                                    ps_main[:, D:D + 1],
                                    lhsT=m_all[:, 2 * jp:2 * jp + 2, :],
                                    rhs=dw_sb[:, to + 2 * jp:
                                              to + 2 * jp + 2, :],
                                    start=jp == 0, stop=jp == np_ - 1,
                                    perf_mode=DoubleRow,
                                )
                        else:
                            for jp in range(np_):
                                mw = m_all[:, 2 * jp:2 * jp + 2, :]
                                st = jp == 0
                                sp = jp == np_ - 1 and sz % 2 == 0
                                for h in range(D // 512):
                                    nc.tensor.matmul(
                                        ps_main[:, h * 512:(h + 1) * 512],
                                        lhsT=mw,
                                        rhs=vg[:, 2 * jp:2 * jp + 2,
                                               h * 512:(h + 1) * 512],
                                        start=st, stop=sp,
                                        perf_mode=DoubleRow,
                                    )
                                nc.tensor.matmul(
                                    ps_main[:, D:D + 1],
                                    lhsT=mw,
                                    rhs=dw_sb[:, to + 2 * jp:
                                              to + 2 * jp + 2, :],
                                    start=st, stop=sp, perf_mode=DoubleRow,
                                )
                        if sz % 2:
                            # odd slot: last k-tile as a plain fp8 matmul
                            mw = m_all[:, sz - 1, :]
                            for h in range(D // 512):
                                nc.tensor.matmul(
                                    ps_main[:, h * 512:(h + 1) * 512],
                                    lhsT=mw,
                                    rhs=vg[:, sz - 1, h * 512:(h + 1) * 512],
                                    start=sz == 1, stop=True,
                                )
                            nc.tensor.matmul(
                                ps_main[:, D:D + 1],
                                lhsT=mw,
                                rhs=dw_sb[:, to + sz - 1, :],
                                start=sz == 1, stop=True,
                            )
                    else:
                        for j in range(sz):
                            mw = m_all[:, j, :]
                            st, sp = j == 0, j == sz - 1
                            for h in range(D // 512):
                                nc.tensor.matmul(
                                    ps_main[:, h * 512:(h + 1) * 512],
                                    lhsT=mw,
                                    rhs=vg[:, j, h * 512:(h + 1) * 512],
                                    start=st, stop=sp,
                                )
                            nc.tensor.matmul(
                                ps_main[:, D:D + 1],
                                lhsT=mw, rhs=ones16,
                                start=st, stop=sp,
                            )
                    # psum -> bf16 stage (DVE, optionally split with Pool),
                    # then store.  Stores default to the SWDGE (gpsimd)
                    # queue -- an HWDGE store waiting on its copies would
                    # head-of-line-block loads (ring) or exps (ACT queue).
                    # The last group has neither problem, so its two halves
                    # can store immediately on the idle HWDGE rings,
                    # shortening the drain tail.
                    HCUT = 512
                    ob = on[:, g * (D + 1):(g + 1) * (D + 1)]
                    last = g == G - 1
                    if STORE_MODE == "swdge2" and not last:
                        if g % 2 == 0:
                            stage2 = stpool.tile([V, 2 * (D + 1)], out_dt)
                        stage_n = stage2[:, (g % 2) * (D + 1):
                                         (g % 2 + 1) * (D + 1)]
                    else:
                        stage_n = stpool.tile([V, D + 1], out_dt)
                    if COPY_SPLIT or (last and LAST_SPLIT):
                        # halves copied on DVE and Pool in parallel; for
                        # the last group each half stores immediately on
                        # its own idle HWDGE ring
                        nc.vector.tensor_copy(stage_n[:, :HCUT],
                                              ps_main[:, :HCUT])
                        if last and LAST_SPLIT:
                            nc.sync.dma_start(ob[:, :HCUT],
                                              stage_n[:, :HCUT])
                        ceng = nc.vector if last else nc.gpsimd
                        ceng.tensor_copy(stage_n[:, HCUT:D + 1],
                                         ps_main[:, HCUT:D + 1])
                        if last and LAST_SPLIT:
                            nc.scalar.dma_start(ob[:, HCUT:D + 1],
                                                stage_n[:, HCUT:D + 1])
                    else:
                        nc.vector.tensor_copy(stage_n, ps_main[:, 0:D + 1])
                    if last:
                        if not LAST_SPLIT:
                            nc.gpsimd.dma_start(ob, stage_n)
                    elif g == G - 2 and STORE_MODE == "swdge2" and g % 2 == 0:
                        # by the time this store's copies finish the load
                        # stream is over -- HWDGE is free and keeps the
                        # SWDGE gen off the Pool queue ahead of the last
                        # group's drain
                        nc.scalar.dma_start(ob, stage_n)
                    elif STORE_MODE == "sync":
                        nc.sync.dma_start(ob, stage_n)
                    elif STORE_MODE == "swdge":
                        nc.gpsimd.dma_start(ob, stage_n)
                    elif STORE_MODE == "swdge2" and (g % 2 == 1 or
                                                     g == G - 2):
                        nc.gpsimd.dma_start(
                            on[:, (g - g % 2) * (D + 1):(g + 1) * (D + 1)],
                            stage2[:, 0:(g % 2 + 1) * (D + 1)])

    nc.compile()
    _BUILD_CACHE[key] = nc
    return nc


# ------------------------------------------------------------------ driver

def _enable_jax_cache():
    """Persistent XLA/NEFF compile cache: a fresh process re-running the
    same geometry skips the ~30s neuronx compile."""
    try:
        import jax

        jax.config.update("jax_compilation_cache_dir", "/tmp/jax_nrt_cache")
        jax.config.update("jax_persistent_cache_min_compile_time_secs", 0.0)
    except Exception:
        pass


def kernel(v_pad, v_len, grid_thws, centers):
    import time as _time

    from concourse.bass_utils import run_bass_kernel_spmd

    _enable_jax_cache()

    v_pad = np.asarray(v_pad)
    v_len = np.asarray(v_len)
    grid_thws = np.asarray(grid_thws)
    centers = np.asarray(centers)

    B, L, D = v_pad.shape
    V = centers.shape[1]

    t0 = _time.monotonic()
    plan = _plan(v_len)
    in_maps = _pack(v_pad, v_len, grid_thws, centers, plan)
    t1 = _time.monotonic()
    nc = _build(plan, D, V, REPLICAS)
    t2 = _time.monotonic()
    res = run_bass_kernel_spmd(nc, in_maps, core_ids=list(range(NCORES)))
    t3 = _time.monotonic()

    G = plan["G"]
    slots = plan["slots"]
    den = np.zeros((B, V), dtype=np.float32)
    num = np.zeros((B, V, D), dtype=np.float32)
    for c in range(NCORES):
        on = np.asarray(res.results[c]["on"], dtype=np.float32)
        for g in range(G):
            slot = slots[c][g]
            if slot is None:
                continue
            b = slot[0]
            blk = on[:, g * (D + 1):(g + 1) * (D + 1)]
            num[b] += blk[:, :D]
            den[b] += blk[:, D]
    out = num / (den + np.float32(1e-6))[:, :, None]
    t4 = _time.monotonic()

    LAST.update(
        plan=plan, nc=nc, res=res,
        pack_s=t1 - t0, build_s=t2 - t1, run_s=t3 - t2, gather_s=t4 - t3,
    )
    return np.ascontiguousarray(out.astype(np.float32))


# revision 53
# speedup vs baseline: 1.0937x; 1.0937x over previous
"""Trainium2 Bass kernel for DynamicViewSampler.

Per sample b (of B=16): spotlight weights m[v,l] = exp(-20*dist2(center_v,
coord_l)) * (l < v_len[b]); out[b,v,:] = (m @ v_pad[b]) / (sum_l m + 1e-6).

Strategy (ragged_sequence): m is exactly 0 for l >= v_len[b], so only
ceil(v_len[b]/128) l-tiles of work exist per sample.  The host packs the
valid 128-row l-tiles into per-core groups (the single SPMD program is
identical across the 8 cores; all per-core variation is carried by the
packed input data).

Three dtype tiers, chosen per sample by v_len (error analysis: the
output is a weighted average over ~0.08*v_len tokens, so quantization
noise in v and m averages out for long samples but shows through for
short ones; measured worst-sample rel err 1.15e-2 vs the 2e-2 gate):
  - LONG (v_len >= 1024): v cast to fp8e4m3, m produced in fp8 by the
    ACT exp, and the numerator matmuls run in DoubleRow perf mode
    (2 fp8 weights per PE cell -> one N=512 matmul contracts TWO 128-row
    l-tiles).  Halves both the HBM traffic and the PE time vs bf16.
  - MID (512 <= v_len < 1024): fp8 m is still fine but fp8 v is not, so
    each l-tile becomes a (hi, lo) fp8 pair (lo = fp8(v - hi), together
    bf16-quality) contracted in one DoubleRow matmul with IDENTICAL m
    columns.  A packed den-weight input (1 for hi, 0 for lo k-tiles)
    replaces the all-ones den rhs so den counts m once -- data-driven,
    so the SPMD program stays identical across cores.
  - SHORT (v_len < 512): bf16 v and m, plain matmuls, packed into
    single-tile groups (so the padding DMA that SPMD forces on the
    other cores stays tiny).

On device, per group (layout: l on partitions):
  - tiny K=4 fp32 matmul per k-tile: psc[l,v] = x_l*cx_v + y_l*cy_v
    - (cx_v^2+cy_v^2)/2 - (x_l^2+y_l^2)/2  (rows: x, y, 1, bias; the
    bias row is -1e5/40 for invalid/padding rows -> m = exp(-1e5) = 0,
    which realizes the ragged mask and all padding).  Kept fp32: short-
    sample softmax gaps are sensitive to coordinate quantization.
  - one batched ACT: m[l,t,v] = Exp(40*psc) -> fp8 (long/mid) / bf16
  - numerator: psum[v,d] += sum_i m[l,2j+i,v].T @ v[l,2j+i,d] via
    DoubleRow pairs; den rides as psum column D (rhs = den weights).
    An odd-sized slot finishes with one plain fp8 matmul.
accumulated in PSUM over the k-tiles of a group (one group = one
contiguous chunk of one sample).  Scheduling: group 0's v-load is issued
first so the HBM stream (the bottleneck: ~13.4us/core of fp8 v data)
starts immediately; cw/dw constants ride the scalar HWDGE ring; psum
drains to a bf16 SBUF stage (DVE) and stores ride the SWDGE queue in
2-group batches, except the last groups which use the by-then-idle HWDGE
rings (the drain tail gates the end-to-end time).  The last group runs
its matmuls h-outer so each psum half can drain while the other half is
still accumulating.  Host sums the per-group partials and divides.
"""

import math

import numpy as np
import ml_dtypes

GAMMA = 20.0
P = 128
NCORES = 8
NEG_BIG = -1e5  # exp(40*psum + NEG_BIG) == 0.0 in fp32 for any |psum| ~ O(1)
FP8_MIN_LEN = 1024   # samples at least this long ride the plain fp8 tier
HILO_MIN_LEN = 512   # [HILO_MIN, FP8_MIN): fp8 hi+lo split (bf16-quality v,
                     # fp8 m); each l-tile becomes a (hi, lo) DoubleRow pair
                     # with den-weights (1, 0) so den counts m only once

# knobs (test.py may override)
REPLICAS = 1          # >1: repeat the whole compute for differential timing
LOOP_N = 1            # >1: wrap the body in a hardware For_i loop (timing)
S8 = 8                # fp8 slot size (even; 8*64*4B = exactly one PSUM bank)
OUT_F32 = False       # numerator partials dtype (bf16 halves out-DMA)
VBUFS = 4             # v-data prefetch depth
COPY_SPLIT = False    # psum->stage copies split DVE+Pool (else DVE only)
STORE_MODE = "swdge2"  # "swdge" | "sync" | "swdge2" (2-group batched)
LAST_SPLIT = True     # last group: two half-stores on the idle HWDGE rings
BF_FIRST = True       # bf16 singleton group first (else last)

LAST = {}             # debug/timing info from the most recent kernel() call

_BUILD_CACHE = {}


# ----------------------------------------------------------------- planning

def _eff_grid(v_len, grid_thws):
    """Replicate reference W_eff/H_eff in float32-exact numpy."""
    Lv = v_len.astype(np.float32)
    H = grid_thws[:, 1].astype(np.float32)
    W = grid_thws[:, 2].astype(np.float32)
    W_eff = np.maximum(1, np.round(np.sqrt(Lv * (W / H))).astype(np.int32))
    H_eff = np.maximum(
        1, np.ceil(Lv / W_eff.astype(np.float32)).astype(np.int32)
    )
    return W_eff, H_eff


def _assign(nt, samples, szs, even_only=None):
    """Best-fit chunks of `samples` (8 core-positions per slot), or None.

    Samples in `even_only` (hilo: k-tiles must pair as hi/lo) may only
    use even-sized slots and always take an even count.
    """
    even_only = even_only or set()
    free = {g: NCORES for g in range(len(szs))}
    placed = {g: [] for g in range(len(szs))}
    order = sorted(samples, key=lambda b: -nt[b])
    for b in order:
        n = int(nt[b])
        ev = b in even_only
        k0 = 0
        while k0 < n:
            rem = n - k0
            ok = [g for g in free if free[g] > 0
                  and not (ev and szs[g] % 2)]
            fits = [g for g in ok if szs[g] >= rem]
            if fits:
                g = min(fits, key=lambda g: (szs[g], g))  # tightest fit
            else:
                if not ok:
                    return None
                g = max(ok, key=lambda g: szs[g])  # biggest, partial
            take = min(szs[g], rem)
            if ev:
                take -= take % 2
            if take == 0:
                return None
            placed[g].append((int(b), k0, take))
            free[g] -= 1
            k0 += take
    out = [[None] * len(szs) for _ in range(NCORES)]
    for g, chunks in placed.items():
        for c, grp in enumerate(chunks):
            out[c][g] = grp
    return out


def _plan(v_len):
    """Choose static per-slot sizes/dtypes and assign sample tile-chunks.

    fp8-tier chunk units are k-tiles: 1 per l-tile for plain-fp8 samples,
    2 per l-tile (hi, lo) for hilo samples.  Both are even-aligned inside
    even-sized slots, so every DoubleRow pair is (2j, 2j+1).
    """
    B = len(v_len)
    nt = np.maximum(1, (np.asarray(v_len).astype(np.int64) + P - 1) // P)
    longs = [b for b in range(B) if v_len[b] >= FP8_MIN_LEN]
    hilos = [b for b in range(B)
             if HILO_MIN_LEN <= v_len[b] < FP8_MIN_LEN]
    shorts = [b for b in range(B) if v_len[b] < HILO_MIN_LEN]
    nt8 = nt.copy()
    nt8[hilos] *= 2  # k-tiles
    f8_samples = longs + hilos
    total8 = int(nt8[f8_samples].sum()) if f8_samples else 0
    total16 = int(nt[shorts].sum()) if shorts else 0

    # fp8 tier: even slot sizes ([S8]*k + an even tail), cheapest feasible
    sizes8, slots8 = [], [[] for _ in range(NCORES)]
    if f8_samples:
        capmin = (total8 + NCORES - 1) // NCORES
        # size multisets from {2..8} with at most one odd slot (an odd
        # slot's last k-tile runs as a single non-DoubleRow matmul) and
        # capacity in [capmin, capmin+8]; cheapest first
        cands = []

        def _gen(prefix, rem, maxsz):
            cap = sum(prefix)
            nodd = sum(s % 2 for s in prefix)
            if nodd > 1:
                return
            if cap >= capmin:
                cands.append((cap * 364 + len(prefix) * 550, list(prefix)))
            if rem == 0 or cap > capmin + S8:
                return
            for s in range(min(S8, maxsz), 1, -1):
                _gen(prefix + [s], rem - 1, s)

        _gen([], 10, S8)
        cands.sort(key=lambda c: c[0])
        for _cost, cand in cands:
            sl = _assign(nt8, f8_samples, cand, even_only=set(hilos))
            if sl is not None:
                sizes8, slots8 = cand, sl
                break
        assert sizes8, "fp8 slot assignment failed"

    # bf16 tier: singleton slots
    n16 = (total16 + NCORES - 1) // NCORES if shorts else 0
    sizes16 = [1] * n16
    slots16 = (_assign(nt, shorts, sizes16) if n16 else
               [[] for _ in range(NCORES)])
    assert slots16 is not None, "bf16 slot assignment failed"

    # program order: one bf16 group first (small first DMA -> PE starts
    # early), then the fp8 groups biggest-first, then remaining bf16.
    groups = []   # (tier, sz, tier_slot_idx)
    nlead = min(1, n16) if BF_FIRST else 0
    if nlead:
        groups.append(("bf", 1, 0))
    order8 = sorted(range(len(sizes8)), key=lambda g: -sizes8[g])
    for g in order8:
        groups.append(("f8", sizes8[g], g))
    for g in range(nlead, n16):
        groups.append(("bf", 1, g))

    G = len(groups)
    slots = [[None] * G for _ in range(NCORES)]
    for c in range(NCORES):
        for gi, (tier, _sz, tg) in enumerate(groups):
            src = slots8 if tier == "f8" else slots16
            if tg < len(src[c]):
                slots[c][gi] = src[c][tg]

    # tile offsets: within each tier's v buffer, and global (for cw cols)
    toff8, toff16, gtoff = [], [], []
    a8 = a16 = at = 0
    for tier, sz, _tg in groups:
        gtoff.append(at)
        at += sz
        if tier == "f8":
            toff8.append(a8)
            a8 += sz
        else:
            toff16.append(a16)
            a16 += sz
    ttoff = []
    i8 = i16 = 0
    for tier, sz, _tg in groups:
        if tier == "f8":
            ttoff.append(toff8[i8]); i8 += 1
        else:
            ttoff.append(toff16[i16]); i16 += 1

    plan = {
        "groups": groups, "slots": slots, "G": G,
        "TT8": a8, "TT16": a16, "TTall": at,
        "ttoff": ttoff, "gtoff": gtoff,
        "total": int(nt.sum()), "total8": total8, "total16": total16,
        "sizes": [sz for _t, sz, _g in groups],
        "hilo": set(hilos),
    }
    return plan


# ------------------------------------------------------------- host packing

def _pack(v_pad, v_len, grid_thws, centers, plan):
    B, L, D = v_pad.shape
    V = centers.shape[1]
    groups, slots, G = plan["groups"], plan["slots"], plan["G"]
    TT8, TT16, TTall = plan["TT8"], plan["TT16"], plan["TTall"]
    ttoff, gtoff = plan["ttoff"], plan["gtoff"]
    W_eff, H_eff = _eff_grid(v_len, grid_thws)

    f8 = ml_dtypes.float8_e4m3
    bf = ml_dtypes.bfloat16
    hilo = plan["hilo"]
    vq = {}  # per-sample quantized v (cast once, shared across cores)

    def getv(b, tier):
        key = (b, tier)
        if key not in vq:
            hi = min(L, int(math.ceil(v_len[b] / P)) * P)
            if tier == "bf":
                vq[key] = v_pad[b, :hi].astype(bf)
            elif b in hilo:
                vhi = v_pad[b, :hi].astype(f8)
                vlo = (v_pad[b, :hi] - vhi.astype(np.float32)).astype(f8)
                vq[key] = (vhi, vlo)
            else:
                vq[key] = v_pad[b, :hi].astype(f8)
        return vq[key]

    in_maps = []
    for c in range(NCORES):
        vp8 = np.zeros(P * TT8 * D, dtype=f8)
        vp16 = np.zeros(P * TT16 * D, dtype=bf)
        # den weights, one column per fp8-tier k-tile: 1 everywhere except
        # the lo halves of hilo pairs (den must count m once per l-tile)
        dw = np.ones((P, max(1, TT8)), dtype=f8)
        cw = np.zeros((4, TTall * P + G * V), dtype=np.float32)
        cw[3, :TTall * P] = np.float32(NEG_BIG / (2 * GAMMA))  # mask default
        cr = cw[:, TTall * P:]
        cr[3, :] = 1.0  # bias row coefficient (also masks dummy groups)
        for g, (tier, sz, _tg) in enumerate(groups):
            slot = slots[c][g]
            if slot is None:
                continue
            b, k0, n_real = slot
            is_hilo = tier == "f8" and b in hilo
            cx = centers[b, :, 0].astype(np.float32)
            cy = centers[b, :, 1].astype(np.float32)
            cr[0, g * V:(g + 1) * V] = cx
            cr[1, g * V:(g + 1) * V] = cy
            cr[2, g * V:(g + 1) * V] = -(cx * cx + cy * cy) / np.float32(2.0)
            We = np.int32(W_eff[b])
            He_f = np.float32(H_eff[b])
            We_f = np.float32(We)
            to = ttoff[g]
            vbuf = vp8 if tier == "f8" else vp16
            block = vbuf[P * to * D:P * (to + sz) * D].reshape(P, sz * D)
            vs = getv(b, tier)
            for j in range(n_real):
                t = gtoff[g] + j
                if is_hilo:
                    k = (k0 + j) // 2       # original l-tile index
                    block[:, j * D:(j + 1) * D] = (
                        vs[(k0 + j) % 2][k * P:(k + 1) * P, :])
                    if (k0 + j) % 2:
                        dw[:, to + j] = 0.0
                else:
                    k = k0 + j
                    block[:, j * D:(j + 1) * D] = vs[k * P:(k + 1) * P, :]
                l = np.arange(k * P, (k + 1) * P, dtype=np.int32)
                x = (l % We).astype(np.float32) / We_f
                y = (l // We).astype(np.float32) / He_f
                cw[0, t * P:(t + 1) * P] = x
                cw[1, t * P:(t + 1) * P] = y
                cw[2, t * P:(t + 1) * P] = 1.0
                valid = l < v_len[b]
                bias = -(x * x + y * y) / np.float32(2.0)
                cw[3, t * P:(t + 1) * P] = np.where(
                    valid, bias.astype(np.float32),
                    np.float32(NEG_BIG / (2 * GAMMA)))
        in_maps.append({"vp8": vp8, "vp16": vp16, "cw": cw, "dw": dw})
    return in_maps


# ------------------------------------------------------------ device kernel

def _build(plan, D, V, replicas):
    groups, G = plan["groups"], plan["G"]
    TT8, TT16, TTall = plan["TT8"], plan["TT16"], plan["TTall"]
    ttoff, gtoff = plan["ttoff"], plan["gtoff"]
    key = (tuple((t, s) for t, s, _g in groups), D, V, replicas, OUT_F32,
           LOOP_N, VBUFS, COPY_SPLIT, STORE_MODE, LAST_SPLIT)
    if key in _BUILD_CACHE:
        return _BUILD_CACHE[key]

    import concourse.bass as bass  # noqa: F401
    import concourse.tile as tile
    from concourse import bacc, mybir

    f32 = mybir.dt.float32
    bf16 = mybir.dt.bfloat16
    fp8 = mybir.dt.float8e4
    out_dt = f32 if OUT_F32 else bf16
    DoubleRow = mybir.MatmulPerfMode.DoubleRow

    nc = bacc.Bacc("TRN2", target_bir_lowering=False, debug=False,
                   num_devices=NCORES)
    vp8 = nc.dram_tensor("vp8", [max(1, P * TT8 * D)], fp8,
                         kind="ExternalInput")
    vp16 = nc.dram_tensor("vp16", [max(1, P * TT16 * D)], bf16,
                          kind="ExternalInput")
    cw = nc.dram_tensor("cw", [4, TTall * P + G * V], f32,
                        kind="ExternalInput")
    dw = nc.dram_tensor("dw", [P, max(1, TT8)], fp8, kind="ExternalInput")
    # den rides as column D of each group's block: one copy, one store
    on = nc.dram_tensor("on", [V, G * (D + 1)], out_dt, kind="ExternalOutput")

    Exp = mybir.ActivationFunctionType.Exp

    with tile.TileContext(nc) as tc:
        with (
            tc.tile_pool(name="singles", bufs=1) as singles,
            tc.tile_pool(name="vpool", bufs=VBUFS) as vpool,
            tc.tile_pool(name="mpool", bufs=3) as mpool,
            tc.tile_pool(name="stage", bufs=3) as stpool,
            tc.tile_pool(name="psc", bufs=2, space="PSUM") as psc,
            tc.tile_pool(name="psm", bufs=2, space="PSUM") as psm,
        ):
            # group 0's v load goes FIRST so the HBM stream (the kernel's
            # critical resource) starts as early as possible; cw/dw slot
            # into the descriptor-generation shadow of the next loads
            tier0, sz0, _tg0 = groups[0]
            vg0 = vpool.tile([P, sz0, D], fp8 if tier0 == "f8" else bf16)
            src0 = (vp8 if tier0 == "f8" else vp16)[
                0:P * sz0 * D].rearrange("(p t d) -> p t d", p=P, t=sz0)
            nc.sync.dma_start(vg0, src0)
            # cw/dw gens ride the scalar ring so the sync ring's gen
            # pipeline stays dedicated to the v-load stream
            cw_sb = singles.tile([4, TTall * P + G * V], f32)
            nc.scalar.dma_start(cw_sb, cw[:, :])
            dw_sb = singles.tile([P, max(1, TT8), 1], fp8)
            nc.scalar.dma_start(dw_sb, dw[:, :].rearrange(
                "p (t one) -> p t one", one=1))
            cr_sb = cw_sb[:, TTall * P:]
            ones16 = singles.tile([P, 1], bf16)
            nc.any.memset(ones16, 1.0)
            # warm the ACT exp table (1.3us load) off the critical path
            warm = singles.tile([1, 1], f32)
            nc.any.memset(warm, 0.0)
            nc.scalar.activation(warm, warm, Exp)

            import contextlib
            loop_ctx = (
                tc.For_i(0, LOOP_N, 1,
                         hint_engines=(mybir.EngineType.PE,
                                       mybir.EngineType.SP,
                                       mybir.EngineType.Activation,
                                       mybir.EngineType.DVE))
                if LOOP_N > 1 else contextlib.nullcontext()
            )
            with loop_ctx:
              for _r in range(replicas):
                for g, (tier, sz, _tg) in enumerate(groups):
                    is8 = tier == "f8"
                    vdt = fp8 if is8 else bf16
                    to = ttoff[g]
                    if g == 0 and _r == 0:
                        vg = vg0
                    else:
                        vg = vpool.tile([P, sz, D], vdt)
                        vsrc = vp8 if is8 else vp16
                        src = vsrc[P * to * D:
                                   P * (to + sz) * D].rearrange(
                            "(p t d) -> p t d", p=P, t=sz)
                        nc.sync.dma_start(vg, src)
                    # one psum tile holds the whole group's coords dots
                    # (bias folded in as the 4th contraction row), exp'd in
                    # a single batched ACT instruction
                    ps_c = psc.tile([P, sz, V], f32)
                    for j in range(sz):
                        t = gtoff[g] + j
                        nc.tensor.matmul(
                            ps_c[:, j, :],
                            lhsT=cw_sb[:, t * P:(t + 1) * P],
                            rhs=cr_sb[:, g * V:(g + 1) * V],
                            start=True, stop=True,
                        )
                    m_all = mpool.tile([P, sz, V], vdt)
                    nc.scalar.activation(m_all, ps_c, Exp, scale=2.0 * GAMMA)
                    # m is the stationary operand: one LDW per (pair of)
                    # tile(s), v streams through as two N=512 matmuls.
                    # num in banks 0-1, den column in bank 2 — three
                    # concurrently-pending accumulation groups in distinct
                    # zero-regions.
                    ps_main = psm.tile([V, D + 1], f32)
                    if is8:
                        np_ = sz // 2
                        if g == G - 1 and sz % 2 == 0:
                            # h-outer for the drain group: the [0:512] psum
                            # bank finishes all its accumulation early, so
                            # its copy+store overlap the second half's
                            # matmuls (costs one extra LDW per pair)
                            for h in range(D // 512):
                                for jp in range(np_):
                                    nc.tensor.matmul(
                                        ps_main[:, h * 512:(h + 1) * 512],
                                        lhsT=m_all[:, 2 * jp:2 * jp + 2, :],
                                        rhs=vg[:, 2 * jp:2 * jp + 2,
                                               h * 512:(h + 1) * 512],
                                        start=jp == 0, stop=jp == np_ - 1,
                                        perf_mode=DoubleRow,
                                    )
                            for jp in range(np_):
                                nc.tensor.matmul(
                                    ps_main[:, D:D + 1],
                                    lhsT=m_all[:, 2 * jp:2 * jp + 2, :],
                                    rhs=dw_sb[:, to + 2 * jp:
                                              to + 2 * jp + 2, :],
                                    start=jp == 0, stop=jp == np_ - 1,
                                    perf_mode=DoubleRow,
                                )
                        else:
                            for jp in range(np_):
                                mw = m_all[:, 2 * jp:2 * jp + 2, :]
                                st = jp == 0
                                sp = jp == np_ - 1 and sz % 2 == 0
                                for h in range(D // 512):
                                    nc.tensor.matmul(
                                        ps_main[:, h * 512:(h + 1) * 512],
                                        lhsT=mw,
                                        rhs=vg[:, 2 * jp:2 * jp + 2,
                                               h * 512:(h + 1) * 512],
                                        start=st, stop=sp,
                                        perf_mode=DoubleRow,
                                    )
                                nc.tensor.matmul(
                                    ps_main[:, D:D + 1],
                                    lhsT=mw,
                                    rhs=dw_sb[:, to + 2 * jp:
                                              to + 2 * jp + 2, :],
                                    start=st, stop=sp, perf_mode=DoubleRow,
                                )
                        if sz % 2:
                            # odd slot: last k-tile as a plain fp8 matmul
                            mw = m_all[:, sz - 1, :]
                            for h in range(D // 512):
                                nc.tensor.matmul(
                                    ps_main[:, h * 512:(h + 1) * 512],
                                    lhsT=mw,
                                    rhs=vg[:, sz - 1, h * 512:(h + 1) * 512],
                                    start=sz == 1, stop=True,
                                )
                            nc.tensor.matmul(
                                ps_main[:, D:D + 1],
                                lhsT=mw,
                                rhs=dw_sb[:, to + sz - 1, :],
                                start=sz == 1, stop=True,
                            )
                    else:
                        for j in range(sz):
                            mw = m_all[:, j, :]
                            st, sp = j == 0, j == sz - 1
                            for h in range(D // 512):
                                nc.tensor.matmul(
                                    ps_main[:, h * 512:(h + 1) * 512],
                                    lhsT=mw,
                                    rhs=vg[:, j, h * 512:(h + 1) * 512],
                                    start=st, stop=sp,
                                )
                            nc.tensor.matmul(
                                ps_main[:, D:D + 1],
                                lhsT=mw, rhs=ones16,
                                start=st, stop=sp,
                            )
                    # psum -> bf16 stage (DVE, optionally split with Pool),
                    # then store.  Stores default to the SWDGE (gpsimd)
                    # queue -- an HWDGE store waiting on its copies would
                    # head-of-line-block loads (ring) or exps (ACT queue).
                    # The last group has neither problem, so its two halves
                    # can store immediately on the idle HWDGE rings,
                    # shortening the drain tail.
                    HCUT = 512
                    ob = on[:, g * (D + 1):(g + 1) * (D + 1)]
                    last = g == G - 1
                    if STORE_MODE == "swdge2" and not last:
                        if g % 2 == 0:
                            stage2 = stpool.tile([V, 2 * (D + 1)], out_dt)
                        stage_n = stage2[:, (g % 2) * (D + 1):
                                         (g % 2 + 1) * (D + 1)]
                    else:
                        stage_n = stpool.tile([V, D + 1], out_dt)
                    if COPY_SPLIT or (last and LAST_SPLIT):
                        # halves copied on DVE and Pool in parallel; for
                        # the last group each half stores immediately on
                        # its own idle HWDGE ring
                        nc.vector.tensor_copy(stage_n[:, :HCUT],
                                              ps_main[:, :HCUT])
                        if last and LAST_SPLIT:
                            nc.sync.dma_start(ob[:, :HCUT],
                                              stage_n[:, :HCUT])
                        ceng = nc.vector if last else nc.gpsimd
                        ceng.tensor_copy(stage_n[:, HCUT:D + 1],
                                         ps_main[:, HCUT:D + 1])
                        if last and LAST_SPLIT:
                            nc.scalar.dma_start(ob[:, HCUT:D + 1],
                                                stage_n[:, HCUT:D + 1])
                    else:
                        nc.vector.tensor_copy(stage_n, ps_main[:, 0:D + 1])
                    if last:
                        if not LAST_SPLIT:
                            nc.gpsimd.dma_start(ob, stage_n)
                    elif g == G - 2 and STORE_MODE == "swdge2" and g % 2 == 0:
                        # by the time this store's copies finish the load
                        # stream is over -- HWDGE is free and keeps the
                        # SWDGE gen off the Pool queue ahead of the last
                        # group's drain
                        nc.scalar.dma_start(ob, stage_n)
                    elif STORE_MODE == "sync":
                        nc.sync.dma_start(ob, stage_n)
                    elif STORE_MODE == "swdge":
                        nc.gpsimd.dma_start(ob, stage_n)
                    elif STORE_MODE == "swdge2" and (g % 2 == 1 or
                                                     g == G - 2):
                        nc.gpsimd.dma_start(
                            on[:, (g - g % 2) * (D + 1):(g + 1) * (D + 1)],
                            stage2[:, 0:(g % 2 + 1) * (D + 1)])

    nc.compile()
    _BUILD_CACHE[key] = nc
    return nc


# ------------------------------------------------------------------ driver

def _enable_jax_cache():
    """Persistent XLA/NEFF compile cache: a fresh process re-running the
    same geometry skips the ~30s neuronx compile."""
    try:
        import jax

        jax.config.update("jax_compilation_cache_dir", "/tmp/jax_nrt_cache")
        jax.config.update("jax_persistent_cache_min_compile_time_secs", 0.0)
    except Exception:
        pass


def kernel(v_pad, v_len, grid_thws, centers):
    import time as _time

    from concourse.bass_utils import run_bass_kernel_spmd

    _enable_jax_cache()

    v_pad = np.asarray(v_pad)
    v_len = np.asarray(v_len)
    grid_thws = np.asarray(grid_thws)
    centers = np.asarray(centers)

    B, L, D = v_pad.shape
    V = centers.shape[1]

    t0 = _time.monotonic()
    plan = _plan(v_len)
    in_maps = _pack(v_pad, v_len, grid_thws, centers, plan)
    t1 = _time.monotonic()
    nc = _build(plan, D, V, REPLICAS)
    t2 = _time.monotonic()
    res = run_bass_kernel_spmd(nc, in_maps, core_ids=list(range(NCORES)))
    t3 = _time.monotonic()

    G = plan["G"]
    slots = plan["slots"]
    den = np.zeros((B, V), dtype=np.float32)
    num = np.zeros((B, V, D), dtype=np.float32)
    for c in range(NCORES):
        on = np.asarray(res.results[c]["on"], dtype=np.float32)
        for g in range(G):
            slot = slots[c][g]
            if slot is None:
                continue
            b = slot[0]
            blk = on[:, g * (D + 1):(g + 1) * (D + 1)]
            num[b] += blk[:, :D]
            den[b] += blk[:, D]
    out = num / (den + np.float32(1e-6))[:, :, None]
    t4 = _time.monotonic()

    LAST.update(
        plan=plan, nc=nc, res=res,
        pack_s=t1 - t0, build_s=t2 - t1, run_s=t3 - t2, gather_s=t4 - t3,
    )
    return np.ascontiguousarray(out.astype(np.float32))
